# revision 1
# baseline (speedup 1.0000x reference)
"""Trainium2 Bass kernel for the LSTM discriminator.

Model: LSTM(H=720) over x[B=2048, T=256, F=51], keep last hidden state,
then sigmoid -> Dense(1024) -> LeakyReLU(0.3) -> Dense(256) -> LeakyReLU(0.3)
-> Dense(1).

Strategy:
  * Data parallel over 8 NeuronCores: 256 batch rows per core; all weights
    replicated.
  * Everything on-device is computed in the *transposed* layout: states and
    activations are [feature, batch] so the per-step recurrence matmul
    z^T = Wh^T h^T + Wx^T x_t^T needs no transposes in the loop (weights in
    natural layout serve directly as lhsT).
  * Per core the 256 batch rows are split into two independent chains of 128.
    The chains are interleaved step by step so the TensorE matmuls of one
    chain overlap the ScalarE (sigmoid/tanh) + VectorE (cell update) work of
    the other chain.
  * Matmuls run in bf16 (1 cycle/row on the PE vs 4 for fp32) with fp32 PSUM
    accumulation; the cell state c stays fp32.
  * The input projection Wx^T x_t is fused into the same PSUM accumulation
    group as the recurrence. x_t^T is zero-padded to K=120 so every matmul
    in the loop has the same contraction size (uniform K avoids a PE
    pipeline hiccup at accumulation-group starts); the LSTM bias rides
    along as a ones-row of x_t^T at partition 96.
  * x arrives in natural [batch, t, feature] layout and is transposed on the
    PE (one 128x51 transpose per chain-step, ~107 ns) via the identity
    matmul trick.
  * The head (sigmoid + 3 dense layers) runs in fp32: bf16 there dominated
    the end-to-end error (the outputs have small magnitude), and the head is
    only ~66 matmuls (~20 us).
  * A ~10 us burst of dummy matmuls right after the weight DMAs warms the
    PE's HAM clock gate to 2.4 GHz before the recurrence starts (otherwise
    the whole loop can run at the cold 1.2 GHz).
"""

import os
import sys

import numpy as np

_TRN = "/opt/trn_rl_repo"
if _TRN not in sys.path:
    sys.path.insert(0, _TRN)

import ml_dtypes  # noqa: E402

import concourse.bacc as bacc  # noqa: E402
import concourse.tile as tile  # noqa: E402
from concourse import mybir  # noqa: E402
from concourse.bass_utils import run_bass_kernel_spmd  # noqa: E402

F32 = mybir.dt.float32
F32R = mybir.dt.float32r
# head matmuls in plain fp32 (4 cycles/row on the PE, but the head is only
# ~66 matmuls so the cost is ~15us; fp32 keeps the head's error ~0 where
# bf16 there dominated the end-to-end error). float32r crashes walrus.
HEAD_DT = F32
BF16 = mybir.dt.bfloat16
AF = mybir.ActivationFunctionType
ALU = mybir.AluOpType

B, T_FULL, F, H = 2048, 256, 51, 720
D1, D2 = 1024, 256
NCORES = 8
BSH = B // NCORES  # 256 batch rows per core
NB = 128           # batch rows per chain (2 chains per core)
HJ, NJ = 120, 6    # H = 720 split into 6 chunks of 120 (partition dim)
G4 = 4 * H         # 2880
KX = 120           # x rows zero-padded to uniform K (=HJ); bias ones-row at ONES_ROW
ONES_ROW = 96
TC = 32            # timesteps of x staged per DMA chunk

_NC_CACHE = {}
LAST_EXEC_NS = None
LAST_RESULTS = None


def _build(T):
    nc = bacc.Bacc(
        "TRN2", target_bir_lowering=False, debug=False, enable_asserts=False
    )

    xa_d = nc.dram_tensor("xa", [NB, T * F], F32, kind="ExternalInput").ap()
    xb_d = nc.dram_tensor("xb", [NB, T * F], F32, kind="ExternalInput").ap()
    wh_d = nc.dram_tensor("wh", [NJ, HJ, G4], BF16, kind="ExternalInput").ap()
    wxb_d = nc.dram_tensor("wxb", [KX, G4], BF16, kind="ExternalInput").ap()
    w1_d = nc.dram_tensor("w1", [NJ, HJ, D1], F32, kind="ExternalInput").ap()
    w2_d = nc.dram_tensor("w2", [8, 128, D2], F32, kind="ExternalInput").ap()
    w3_d = nc.dram_tensor("w3", [2, 128, 1], F32, kind="ExternalInput").ap()
    b1_d = nc.dram_tensor("b1t", [128, 8], F32, kind="ExternalInput").ap()
    b2_d = nc.dram_tensor("b2t", [128, 2], F32, kind="ExternalInput").ap()
    b3_d = nc.dram_tensor("b3t", [1, 1], F32, kind="ExternalInput").ap()
    id_d = nc.dram_tensor("ident", [128, 128], F32, kind="ExternalInput").ap()
    out_d = nc.dram_tensor("out", [BSH, 1], F32, kind="ExternalOutput").ap()

    import contextlib

    with tile.TileContext(nc) as tc, contextlib.ExitStack() as ctx:
        if T > 16:
            tc.race_detector_enabled = False

        const = ctx.enter_context(tc.tile_pool(name="const", bufs=1))
        xpool = ctx.enter_context(tc.tile_pool(name="xp", bufs=2))
        gpool = ctx.enter_context(tc.tile_pool(name="gp", bufs=3))
        spool = ctx.enter_context(tc.tile_pool(name="st", bufs=1))
        zpool = ctx.enter_context(tc.tile_pool(name="zp", bufs=6, space="PSUM"))
        tpool = ctx.enter_context(tc.tile_pool(name="tp", bufs=2, space="PSUM"))

        # ---- weights / constants into SBUF ----
        # DMA order matters: identity + x chunks first (the warmup matmuls
        # and first transposes depend on them), then the LSTM weights, then
        # head weights (needed only at the very end).
        id_t = const.tile([128, 128], F32, tag="id", name="id")
        nc.sync.dma_start(id_t[:], id_d[:])
        wxb_t = const.tile([KX, G4], BF16, tag="wxb", name="wxb")
        wh_t = const.tile([HJ, NJ * G4], BF16, tag="wh", name="wh")
        w1_t = const.tile([HJ, NJ * D1], F32, tag="w1", name="w1")
        w2_t = const.tile([128, 8 * D2], F32, tag="w2", name="w2")
        w3_t = const.tile([128, 2], F32, tag="w3", name="w3")
        b1_t = const.tile([128, 8], F32, tag="b1", name="b1")
        b2_t = const.tile([128, 2], F32, tag="b2", name="b2")
        b3_t = const.tile([1, 1], F32, tag="b3", name="b3")

        def load_weights():
            # spread the big weight loads across several engines' DMA queues
            # so they run in parallel instead of serializing behind x
            qs = [nc.gpsimd, nc.scalar, nc.sync]
            qi = [0]

            def dma(dst, src):
                qs[qi[0] % len(qs)].dma_start(dst, src)
                qi[0] += 1

            dma(wxb_t[:], wxb_d[:])
            for j in range(NJ):
                dma(wh_t[:, j * G4 : (j + 1) * G4], wh_d[j])
            for j in range(NJ):
                dma(w1_t[:, j * D1 : (j + 1) * D1], w1_d[j])
            for k in range(8):
                dma(w2_t[:, k * D2 : (k + 1) * D2], w2_d[k])
            for k in range(2):
                dma(w3_t[:, k : k + 1], w3_d[k])
            dma(b1_t[:], b1_d[:])
            dma(b2_t[:], b2_d[:])
            dma(b3_t[:], b3_d[:])

        # ---- persistent state ----
        # h, c, in transposed layout: [HJ, NJ * NB]; column block j holds
        # feature rows [120j, 120j+120) for the chain's 128 batch cols.
        hT = [
            [spool.tile([HJ, NJ * NB], BF16, tag=f"h{c}{p}", name=f"h{c}{p}") for p in range(2)]
            for c in range(2)
        ]
        cT = [spool.tile([HJ, NJ * NB], F32, tag=f"c{c}", name=f"c{c}") for c in range(2)]
        xT = [spool.tile([KX, NB], BF16, tag=f"xT{c}", name=f"xT{c}") for c in range(2)]
        for c in range(2):
            nc.vector.memset(hT[c][0][:], 0.0)
            nc.vector.memset(cT[c][:], 0.0)
            # rows 0..F-1 are overwritten by the per-step transpose copy;
            # rows F..KX-1 stay 0 except the bias ones-row at ONES_ROW.
            # Zero-padding x to K=120 keeps every matmul in the loop at the
            # same contraction size (no PE pipeline disruption at group
            # starts from K changes).
            nc.vector.memset(xT[c][:], 0.0)
            nc.vector.memset(xT[c][ONES_ROW : ONES_ROW + 1, :], 1.0)

        xd = [xa_d, xb_d]
        nchunks = (T + TC - 1) // TC
        xtiles = [[None] * nchunks for _ in range(2)]

        def ensure_chunk(c, ch):
            if ch >= nchunks or xtiles[c][ch] is not None:
                return
            sz = min(TC, T - ch * TC)
            t_ = xpool.tile([NB, TC * F], F32, tag=f"xc{c}", name=f"xc{c}")
            nc.sync.dma_start(
                t_[:, : sz * F], xd[c][:, ch * TC * F : (ch * TC + sz) * F]
            )
            xtiles[c][ch] = t_

        ensure_chunk(0, 0)
        ensure_chunk(1, 0)
        load_weights()

        def emit_transpose(s):
            c, t = s % 2, s // 2
            ch, off = t // TC, t % TC
            ensure_chunk(c, ch)
            if off == 0:
                ensure_chunk(c, ch + 1)  # prefetch the next chunk early
            tp = tpool.tile([F, NB], F32, tag="tp", name="tp")
            nc.tensor.transpose(
                tp[:], xtiles[c][ch][:, off * F : (off + 1) * F], id_t[:]
            )
            nc.vector.tensor_copy(xT[c][0:F, :], tp[:])

        def emit_chain_step(s):
            c, t = s % 2, s // 2
            p = t % 2
            h_rd = hT[c][p]
            h_wr = hT[c][1 - p]
            for jlist in ((0, 1, 2, 3), (4, 5)):
                W = NB * len(jlist)
                c0 = NB * jlist[0]
                zt = []
                for g in range(4):
                    z = zpool.tile([HJ, W], F32, tag="z", name="z")
                    for ii, j in enumerate(jlist):
                        mc = 720 * g + HJ * j
                        o0 = ii * NB
                        nc.tensor.matmul(
                            z[:, o0 : o0 + NB],
                            wxb_t[:, mc : mc + HJ],
                            xT[c][:],
                            start=True,
                            stop=False,
                        )
                        for k in range(NJ):
                            nc.tensor.matmul(
                                z[:, o0 : o0 + NB],
                                wh_t[:, k * G4 + mc : k * G4 + mc + HJ],
                                h_rd[:, k * NB : (k + 1) * NB],
                                start=False,
                                stop=(k == NJ - 1),
                            )
                    zt.append(z)
                si = gpool.tile([HJ, W], F32, tag="si", name="si")
                nc.scalar.activation(si[:], zt[0][:], AF.Sigmoid)
                sf = gpool.tile([HJ, W], F32, tag="sf", name="sf")
                nc.scalar.activation(sf[:], zt[1][:], AF.Sigmoid)
                tg = gpool.tile([HJ, W], F32, tag="tg", name="tg")
                nc.scalar.activation(tg[:], zt[2][:], AF.Tanh)
                so = gpool.tile([HJ, W], F32, tag="so", name="so")
                nc.scalar.activation(so[:], zt[3][:], AF.Sigmoid)
                csl = cT[c][:, c0 : c0 + W]
                t1 = gpool.tile([HJ, W], F32, tag="t1", name="t1")
                nc.vector.tensor_mul(t1[:], sf[:], csl)
                t2 = gpool.tile([HJ, W], F32, tag="t2", name="t2")
                nc.vector.tensor_mul(t2[:], si[:], tg[:])
                nc.vector.tensor_add(csl, t1[:], t2[:])
                tq = gpool.tile([HJ, W], F32, tag="tc", name="tc")
                nc.scalar.activation(tq[:], csl, AF.Tanh)
                nc.vector.tensor_mul(h_wr[:, c0 : c0 + W], so[:], tq[:])

        # HAM warmup: ~6us of dense matmul work reading wh (the last big DMA),
        # so it runs right before the loop with no idle gap and flips the PE
        # clock gate to 8/8 (2.4 GHz) before the recurrence starts.
        # (reads the LAST wh block so it starts only after all LSTM weights
        # are resident and runs flush against the loop start)
        wm = zpool.tile([128, 512], F32, tag="z", name="wm")
        w0 = (NJ - 1) * G4
        for w_ in range(16):
            nc.tensor.matmul(
                wm[:],
                wh_t[:, w0 : w0 + 128],
                wh_t[:, w0 + 1024 : w0 + 1536],
                start=True,
                stop=True,
            )

        emit_transpose(0)
        S = 2 * T
        for s in range(S):
            if s + 1 < S:
                emit_transpose(s + 1)
            emit_chain_step(s)

        # ---- head: sigmoid -> FC1+leaky -> FC2+leaky -> FC3 ----
        pfin = T % 2
        sgh = spool.tile([HJ, NJ * BSH], F32, tag="sgh", name="sgh")
        for j in range(NJ):
            for c in range(2):
                d0 = j * BSH + c * NB
                nc.scalar.activation(
                    sgh[:, d0 : d0 + NB],
                    hT[c][pfin][:, j * NB : (j + 1) * NB],
                    AF.Sigmoid,
                )
        o1 = spool.tile([128, 8 * BSH], F32, tag="o1", name="o1")
        for m in range(8):
            ps = zpool.tile([128, BSH], F32, tag="z", name="z")
            for j in range(NJ):
                nc.tensor.matmul(
                    ps[:],
                    w1_t[:, j * D1 + m * 128 : j * D1 + (m + 1) * 128].bitcast(HEAD_DT),
                    sgh[:, j * BSH : (j + 1) * BSH].bitcast(HEAD_DT),
                    start=(j == 0),
                    stop=(j == NJ - 1),
                )
            tb = gpool.tile([128, BSH], F32, tag="hb", name="hb")
            nc.vector.tensor_scalar_add(tb[:], ps[:], b1_t[:, m : m + 1])
            nc.vector.scalar_tensor_tensor(
                o1[:, m * BSH : (m + 1) * BSH], tb[:], 0.3, tb[:], ALU.mult, ALU.max
            )
        o2 = spool.tile([128, 2 * BSH], F32, tag="o2", name="o2")
        for m in range(2):
            ps = zpool.tile([128, BSH], F32, tag="z", name="z")
            for k in range(8):
                nc.tensor.matmul(
                    ps[:],
                    w2_t[:, k * D2 + m * 128 : k * D2 + (m + 1) * 128].bitcast(HEAD_DT),
                    o1[:, k * BSH : (k + 1) * BSH].bitcast(HEAD_DT),
                    start=(k == 0),
                    stop=(k == 7),
                )
            tb = gpool.tile([128, BSH], F32, tag="hb", name="hb")
            nc.vector.tensor_scalar_add(tb[:], ps[:], b2_t[:, m : m + 1])
            nc.vector.scalar_tensor_tensor(
                o2[:, m * BSH : (m + 1) * BSH], tb[:], 0.3, tb[:], ALU.mult, ALU.max
            )
        ps = zpool.tile([1, BSH], F32, tag="z", name="z")
        for k in range(2):
            nc.tensor.matmul(
                ps[:],
                w3_t[:, k : k + 1].bitcast(HEAD_DT),
                o2[:, k * BSH : (k + 1) * BSH].bitcast(HEAD_DT),
                start=(k == 0),
                stop=(k == 1),
            )
        ob = spool.tile([1, BSH], F32, tag="ob", name="ob")
        nc.vector.tensor_scalar_add(ob[:], ps[:], b3_t[:])
        nc.sync.dma_start(out_d[:], ob[:])

    nc.compile()
    return nc


def _get_nc(T):
    if T not in _NC_CACHE:
        _NC_CACHE[T] = _build(T)
    return _NC_CACHE[T]


def kernel(x, Wx, Wh, b, W1, b1, W2, b2, W3, b3):
    global LAST_EXEC_NS, LAST_RESULTS
    x = np.asarray(x, dtype=np.float32)
    T = x.shape[1]
    nc = _get_nc(T)

    bf = ml_dtypes.bfloat16
    wh = np.ascontiguousarray(np.asarray(Wh, np.float32).reshape(NJ, HJ, G4)).astype(bf)
    wxb = np.zeros((KX, G4), np.float32)
    wxb[:F] = np.asarray(Wx, np.float32)
    wxb[ONES_ROW] = np.asarray(b, np.float32)
    wxb = wxb.astype(bf)
    w1 = np.ascontiguousarray(np.asarray(W1, np.float32).reshape(NJ, HJ, D1))
    w2 = np.ascontiguousarray(np.asarray(W2, np.float32).reshape(8, 128, D2))
    w3 = np.ascontiguousarray(np.asarray(W3, np.float32).reshape(2, 128, 1))
    b1t = np.ascontiguousarray(np.asarray(b1, np.float32).reshape(8, 128).T)
    b2t = np.ascontiguousarray(np.asarray(b2, np.float32).reshape(2, 128).T)
    b3t = np.asarray(b3, np.float32).reshape(1, 1)
    ident = np.eye(128, dtype=np.float32)

    shared = {
        "wh": wh,
        "wxb": wxb,
        "w1": w1,
        "w2": w2,
        "w3": w3,
        "b1t": b1t,
        "b2t": b2t,
        "b3t": b3t,
        "ident": ident,
    }
    in_maps = []
    for i in range(NCORES):
        xs = x[i * BSH : (i + 1) * BSH]
        in_maps.append(
            {
                "xa": np.ascontiguousarray(xs[:NB].reshape(NB, T * F)),
                "xb": np.ascontiguousarray(xs[NB:].reshape(NB, T * F)),
                **shared,
            }
        )

    trace = bool(os.environ.get("KLSTM_TRACE"))
    res = run_bass_kernel_spmd(nc, in_maps, list(range(NCORES)), trace=trace)
    LAST_RESULTS = res
    LAST_EXEC_NS = res.exec_time_ns
    out = np.concatenate([r["out"] for r in res.results], axis=0)
    return out.astype(np.float32)



# revision 7
# speedup vs baseline: 1.0852x; 1.0852x over previous
"""Trainium2 Bass kernel for the LSTM discriminator (fp8 DoubleRow version).

Model: LSTM(H=720) over x[B=2048, T=256, F=51], keep last hidden state,
then sigmoid -> Dense(1024) -> LeakyReLU(0.3) -> Dense(256) -> LeakyReLU(0.3)
-> Dense(1).

Strategy vs the bf16 baseline (which ran at the bf16 PE roofline, ~4.94ms):
  * Recurrence matmuls in fp8-e4m3 with MatmulPerfMode.DoubleRow: one
    instruction contracts TWO 128-row K-tiles (2x bf16 throughput on the PE).
    DoubleRow only pays off when the moving operand is wide, so z is computed
    *batch-major*: out[z^T] = [128 batch, 2880 gates], stationary = fp8
    h-state pairs [128, 2, 128], moving = fp8 Wh pairs [128, 2, <=512].
  * Quantization: Wh as e4m3(64*Wh), h as e4m3(16*h); the x projection stays
    bf16 (x-path quantization dominated the fp8 error budget) with weights
    pre-scaled by 1024 so both parts share one PSUM accumulation; the gate
    activation applies scale=1/1024. Measured end-to-end rel err ~3e-3.
  * x is pre-transposed on the host to [52, T, 128] bf16 per chain (51
    features + a ones-row that carries the LSTM bias), so the inner loop has
    no x transposes at all.
  * Two interleaved chains of 128 batch rows per core; z PSUM is 3 shared
    [128, 1024] tiles (6 banks) alternating between chains - ACT ops are
    tile-aligned so the write-after-read pacing is tile-granular.
  * Gate order in z columns is [i, f, o, g]: one sigmoid span (2160 cols) and
    one tanh span (720). Pointwise work is spread over ACT (gates + tanh(c)),
    GPSIMD (f*c, i*g) and DVE (c update, h=o*tanh(c), fp8 h-transpose copy).
  * h goes back to feature-major for the next step's stationary via 6 PE
    transposes (bf16, one PSUM bank), emitted mid-way through the *next*
    slot's matmul stream so the PE never waits on the pointwise chain.
  * fp32 head (sigmoid -> 3 dense layers), ~30us, after the loop.
"""

import os
import sys

import numpy as np

_TRN = "/opt/trn_rl_repo"
if _TRN not in sys.path:
    sys.path.insert(0, _TRN)

import ml_dtypes  # noqa: E402

import concourse.bacc as bacc  # noqa: E402
import concourse.tile as tile  # noqa: E402
from concourse import mybir  # noqa: E402
from concourse.bass_utils import run_bass_kernel_spmd  # noqa: E402

F32 = mybir.dt.float32
BF16 = mybir.dt.bfloat16
FP8 = mybir.dt.float8e4
AF = mybir.ActivationFunctionType
ALU = mybir.AluOpType
PM = mybir.MatmulPerfMode.DoubleRow

B, T_FULL, F, H = 2048, 256, 51, 720
D1, D2 = 1024, 256
NCORES = 8
BSH = B // NCORES   # 256 batch rows per core
NB = 128            # batch rows per chain (2 chains per core)
KX = F + 1          # x rows + ones row (bias)
G4 = 4 * H          # 2880
NP = 6              # h feature planes of 128 (720 -> 5x128 + 80, zero-padded)
TC = 32             # timesteps of x staged per DMA chunk
TPR = 4             # z-range index where the prev slot's h transposes go
S_W = 64.0          # Wh fp8 scale
S_H = 16.0          # h fp8 scale
S_Z = S_W * S_H     # total z scale (x-path weights pre-scaled by this)

# z column ranges (gate order i, f, o, g)
# sigmoid: cols [0, 2160), tanh(g): cols [2160, 2880)
ZW = [512, 512, 512, 512, 512, 320]   # 6 ranges over 3 [128,1024] tiles

_NC_CACHE = {}
LAST_EXEC_NS = None
LAST_RESULTS = None


def _build(T):
    nc = bacc.Bacc(
        "TRN2", target_bir_lowering=False, debug=False, enable_asserts=False
    )

    xd = [
        nc.dram_tensor(f"x{c}", [KX, T, NB], BF16, kind="ExternalInput").ap()
        for c in range(2)
    ]
    whp_d = nc.dram_tensor("whp", [128, 3, 2, G4], FP8, kind="ExternalInput").ap()
    wxb_d = nc.dram_tensor("wxb", [KX, G4], BF16, kind="ExternalInput").ap()
    w1_d = nc.dram_tensor("w1", [128, NP, D1], F32, kind="ExternalInput").ap()
    w2_d = nc.dram_tensor("w2", [128, 8, D2], F32, kind="ExternalInput").ap()
    w3_d = nc.dram_tensor("w3", [128, 2, 1], F32, kind="ExternalInput").ap()
    b1_d = nc.dram_tensor("b1t", [128, 8], F32, kind="ExternalInput").ap()
    b2_d = nc.dram_tensor("b2t", [128, 2], F32, kind="ExternalInput").ap()
    b3_d = nc.dram_tensor("b3t", [1, 1], F32, kind="ExternalInput").ap()
    idb_d = nc.dram_tensor("identb", [128, 128], BF16, kind="ExternalInput").ap()
    idf_d = nc.dram_tensor("identf", [128, 128], F32, kind="ExternalInput").ap()
    out_d = nc.dram_tensor("out", [BSH, 1], F32, kind="ExternalOutput").ap()

    import contextlib

    with tile.TileContext(nc) as tc, contextlib.ExitStack() as ctx:
        if T > 16:
            tc.race_detector_enabled = False

        const = ctx.enter_context(tc.tile_pool(name="const", bufs=1))
        xpool = ctx.enter_context(tc.tile_pool(name="xp", bufs=2))
        gpool = ctx.enter_context(tc.tile_pool(name="gp", bufs=2))
        spool = ctx.enter_context(tc.tile_pool(name="st", bufs=1))
        zpool = ctx.enter_context(tc.tile_pool(name="zp", bufs=1, space="PSUM"))
        tpool = ctx.enter_context(tc.tile_pool(name="tp", bufs=1, space="PSUM"))
        hpool = ctx.enter_context(tc.tile_pool(name="hp", bufs=1, space="PSUM"))

        # ---- constants / weights ----
        idb_t = const.tile([128, 128], BF16, tag="idb", name="idb")
        nc.sync.dma_start(idb_t[:], idb_d[:])
        idf_t = const.tile([128, 128], F32, tag="idf", name="idf")
        nc.sync.dma_start(idf_t[:], idf_d[:])
        whp_t = const.tile([128, 3, 2, G4], FP8, tag="whp", name="whp")
        wxb_t = const.tile([KX, G4], BF16, tag="wxb", name="wxb")
        w1_t = const.tile([128, NP, D1], F32, tag="w1", name="w1")
        w2_t = const.tile([128, 8, D2], F32, tag="w2", name="w2")
        w3_t = const.tile([128, 2, 1], F32, tag="w3", name="w3")
        b1_t = const.tile([128, 8], F32, tag="b1", name="b1")
        b2_t = const.tile([128, 2], F32, tag="b2", name="b2")
        b3_t = const.tile([1, 1], F32, tag="b3", name="b3")

        def load_weights():
            qs = [nc.gpsimd, nc.scalar, nc.sync]
            qi = [0]

            def dma(dst, src):
                qs[qi[0] % len(qs)].dma_start(dst, src)
                qi[0] += 1

            for k in range(NP):
                dma(w1_t[:, k, :], w1_d[:, k, :])
            for k in range(8):
                dma(w2_t[:, k, :], w2_d[:, k, :])
            dma(w3_t[:], w3_d[:])
            dma(b1_t[:], b1_d[:])
            dma(b2_t[:], b2_d[:])
            dma(b3_t[:], b3_d[:])
            dma(wxb_t[:], wxb_d[:])
            # whp last: the warmup matmuls read it, so they start right after
            # the weight DMAs and ramp the PE clock before the loop
            for p in range(3):
                dma(whp_t[:, p, :, :], whp_d[:, p, :, :])

        # ---- persistent state ----
        # hTq: fp8 16*h, feature-major planes [128 feat, plane, 128 batch]
        hTq = [spool.tile([128, NP, NB], FP8, tag=f"hTq{c}", name=f"hTq{c}") for c in range(2)]
        cS = [spool.tile([NB, H], F32, tag=f"c{c}", name=f"c{c}") for c in range(2)]
        # gates (f32, batch-major): sig = [i | f | o], tg = tanh(g)
        sig = [spool.tile([NB, 2160], F32, tag=f"sg{c}", name=f"sg{c}") for c in range(2)]
        tg = [spool.tile([NB, H], F32, tag=f"tg{c}", name=f"tg{c}") for c in range(2)]
        tcl = [spool.tile([NB, H], F32, tag=f"tc{c}", name=f"tc{c}") for c in range(2)]
        t1g = [spool.tile([NB, H], F32, tag=f"t1{c}", name=f"t1{c}") for c in range(2)]
        t2g = [spool.tile([NB, H], F32, tag=f"t2{c}", name=f"t2{c}") for c in range(2)]
        hb = [spool.tile([NB, H], BF16, tag=f"hb{c}", name=f"hb{c}") for c in range(2)]
        for c in range(2):
            nc.vector.memset(hTq[c][:], 0.0)
            nc.vector.memset(cS[c][:], 0.0)

        # shared z PSUM: 3 tiles [128, 1024] = 6 banks, alternating chains
        zt = [zpool.tile([NB, 1024], F32, tag=f"z{i}", name=f"z{i}") for i in range(3)]
        # h-transpose landing bank (bf16, 6 planes); rows >= 80 of plane 5 are
        # never written by the transposes - zero once so the hTq copy reads 0s
        # (fp8 garbage there could be NaN patterns, and NaN*0 = NaN in PSUM)
        tpt = tpool.tile([128, NP, NB], BF16, tag="tpt", name="tpt")
        nc.vector.memset(tpt[:].bitcast(F32), 0.0)

        # ---- x chunks ----
        nchunks = (T + TC - 1) // TC
        xtiles = [[None] * nchunks for _ in range(2)]

        def ensure_chunk(c, ch):
            if ch >= nchunks or xtiles[c][ch] is not None:
                return
            sz = min(TC, T - ch * TC)
            t_ = xpool.tile([KX, TC, NB], BF16, tag=f"xc{c}", name=f"xc{c}")
            nc.sync.dma_start(t_[:, :sz, :], xd[c][:, ch * TC : ch * TC + sz, :])
            xtiles[c][ch] = t_

        ensure_chunk(0, 0)
        ensure_chunk(1, 0)
        load_weights()

        # ---- HAM warmup: PE busy ~5us right after the whp DMA lands ----
        for w_ in range(10):
            nc.tensor.matmul(
                zt[0][:, 0:512],
                hTq[0][:, 0:2, :],
                whp_t[:, 2, :, 2368:2880],
                start=True,
                stop=True,
                perf_mode=PM,
            )

        def emit_z(s):
            c, t = s % 2, s // 2
            ch, toff = t // TC, t % TC
            if toff == 0:
                ensure_chunk(c, ch + 1)
            xsl = xtiles[c][ch][:, toff, :]
            col = 0
            for r in range(6):
                w = ZW[r]
                ztile = zt[r // 2]
                o0 = (r % 2) * 512
                out = ztile[:, o0 : o0 + w]
                nc.tensor.matmul(
                    out, xsl, wxb_t[:, col : col + w], start=True, stop=False
                )
                for p in range(3):
                    nc.tensor.matmul(
                        out,
                        hTq[c][:, 2 * p : 2 * p + 2, :],
                        whp_t[:, p, :, col : col + w],
                        start=False,
                        stop=(p == 2),
                        perf_mode=PM,
                    )
                col += w
                if r == TPR and s >= 1:
                    emit_htranspose(s - 1)

        def emit_htranspose(s):
            # transpose h of slot s (bf16) into tpt, then fp8*16 into hTq
            c = s % 2
            for j in range(NP):
                w = min(128, H - 128 * j)
                nc.tensor.transpose(
                    tpt[0:w, j, :], hb[c][:, 128 * j : 128 * j + w], idb_t[:]
                )
            nc.vector.tensor_scalar_mul(hTq[c][:], tpt[:], S_H)

        def emit_pointwise(s):
            c, t = s % 2, s // 2
            # gates: z tiles -> SBUF, descaled by 1/S_Z
            nc.scalar.activation(sig[c][:, 0:1024], zt[0][:], AF.Sigmoid, scale=1.0 / S_Z)
            nc.scalar.activation(
                sig[c][:, 1024:2048], zt[1][:], AF.Sigmoid, scale=1.0 / S_Z
            )
            nc.scalar.activation(
                sig[c][:, 2048:2160], zt[2][:, 0:112], AF.Sigmoid, scale=1.0 / S_Z
            )
            nc.scalar.activation(
                tg[c][:], zt[2][:, 112:832], AF.Tanh, scale=1.0 / S_Z
            )
            # cell update
            nc.gpsimd.tensor_mul(t1g[c][:], sig[c][:, 720:1440], cS[c][:])
            nc.gpsimd.tensor_mul(t2g[c][:], sig[c][:, 0:720], tg[c][:])
            nc.vector.tensor_add(cS[c][:], t1g[c][:], t2g[c][:])
            nc.scalar.activation(tcl[c][:], cS[c][:], AF.Tanh)
            if t < T - 1:
                nc.vector.tensor_mul(hb[c][:], sig[c][:, 1440:2160], tcl[c][:])

        S = 2 * T
        for s in range(S):
            emit_z(s)
            emit_pointwise(s)

        # ---- head ----
        sgT = spool.tile([128, NP, BSH], F32, tag="sgT", name="sgT")
        hf = spool.tile([NB, H], F32, tag="hf", name="hf")
        for c in range(2):
            nc.vector.tensor_mul(hf[:], sig[c][:, 1440:2160], tcl[c][:])
            nc.scalar.activation(hf[:], hf[:], AF.Sigmoid)
            for j in range(NP):
                w = min(128, H - 128 * j)
                nc.tensor.transpose(
                    zt[0].bitcast(F32)[0:w, 128 * j : 128 * j + 128],
                    hf[:, 128 * j : 128 * j + w],
                    idf_t[:],
                )
            nc.vector.tensor_copy(
                sgT[:, :, c * NB : c * NB + NB],
                zt[0][:, 0:768],
            )

        o1 = spool.tile([128, 8, BSH], F32, tag="o1", name="o1")
        for m in range(8):
            ps = hpool.tile([128, BSH], F32, tag="hps", name="hps")
            for j in range(NP):
                nc.tensor.matmul(
                    ps[:],
                    w1_t[:, j, m * 128 : (m + 1) * 128],
                    sgT[:, j, :],
                    start=(j == 0),
                    stop=(j == NP - 1),
                )
            tb = gpool.tile([128, BSH], F32, tag="hb2", name="hb2")
            nc.vector.tensor_scalar_add(tb[:], ps[:], b1_t[:, m : m + 1])
            nc.vector.scalar_tensor_tensor(
                o1[:, m, :], tb[:], 0.3, tb[:], ALU.mult, ALU.max
            )
        o2 = spool.tile([128, 2, BSH], F32, tag="o2", name="o2")
        for m in range(2):
            ps = hpool.tile([128, BSH], F32, tag="hps", name="hps")
            for k in range(8):
                nc.tensor.matmul(
                    ps[:],
                    w2_t[:, k, m * 128 : (m + 1) * 128],
                    o1[:, k, :],
                    start=(k == 0),
                    stop=(k == 7),
                )
            tb = gpool.tile([128, BSH], F32, tag="hb2", name="hb2")
            nc.vector.tensor_scalar_add(tb[:], ps[:], b2_t[:, m : m + 1])
            nc.vector.scalar_tensor_tensor(
                o2[:, m, :], tb[:], 0.3, tb[:], ALU.mult, ALU.max
            )
        ps3 = hpool.tile([128, BSH], F32, tag="hps", name="hps3")
        ps = ps3[0:1, :]
        for k in range(2):
            nc.tensor.matmul(
                ps, w3_t[:, k, :], o2[:, k, :], start=(k == 0), stop=(k == 1)
            )
        ob = spool.tile([1, BSH], F32, tag="ob", name="ob")
        nc.vector.tensor_scalar_add(ob[:], ps, b3_t[:])
        nc.sync.dma_start(out_d[:], ob[:])

    nc.compile()
    return nc


def _get_nc(T):
    if T not in _NC_CACHE:
        _NC_CACHE[T] = _build(T)
    return _NC_CACHE[T]


def _prep_weights(Wx, Wh, b, W1, b1, W2, b2, W3, b3):
    f32 = np.float32
    bf = ml_dtypes.bfloat16
    e4 = ml_dtypes.float8_e4m3fn

    # gate reorder: reference z = [i | f | g | o] -> ours [i | f | o | g]
    def reord(w):
        return np.concatenate(
            [w[..., 0:1440], w[..., 2160:2880], w[..., 1440:2160]], axis=-1
        )

    Whr = reord(np.asarray(Wh, f32))
    Wxr = reord(np.asarray(Wx, f32))
    br = reord(np.asarray(b, f32).reshape(1, G4))[0]

    whp = np.zeros((128, 3, 2, G4), f32)
    for p in range(3):
        for q in range(2):
            r0 = 128 * (2 * p + q)
            r1 = min(r0 + 128, H)
            if r0 < H:
                whp[0 : r1 - r0, p, q, :] = Whr[r0:r1]
    whp = np.clip(whp * S_W, -240, 240).astype(e4)

    wxb = np.zeros((KX, G4), f32)
    wxb[:F] = Wxr
    wxb[F] = br
    wxb = (wxb * S_Z).astype(bf)

    w1 = np.zeros((128, NP, D1), f32)
    W1a = np.asarray(W1, f32)
    for j in range(NP):
        r0 = 128 * j
        r1 = min(r0 + 128, H)
        w1[0 : r1 - r0, j, :] = W1a[r0:r1]
    w2 = np.ascontiguousarray(np.asarray(W2, f32).reshape(8, 128, D2).transpose(1, 0, 2))
    w3 = np.ascontiguousarray(np.asarray(W3, f32).reshape(2, 128, 1).transpose(1, 0, 2))
    b1t = np.ascontiguousarray(np.asarray(b1, f32).reshape(8, 128).T)
    b2t = np.ascontiguousarray(np.asarray(b2, f32).reshape(2, 128).T)
    b3t = np.asarray(b3, f32).reshape(1, 1)
    return {
        "whp": whp,
        "wxb": wxb,
        "w1": w1,
        "w2": w2,
        "w3": w3,
        "b1t": b1t,
        "b2t": b2t,
        "b3t": b3t,
        "identb": np.eye(128, dtype=bf),
        "identf": np.eye(128, dtype=f32),
    }


def kernel(x, Wx, Wh, b, W1, b1, W2, b2, W3, b3):
    global LAST_EXEC_NS, LAST_RESULTS
    x = np.asarray(x, dtype=np.float32)
    T = x.shape[1]
    nc = _get_nc(T)
    bf = ml_dtypes.bfloat16

    shared = _prep_weights(Wx, Wh, b, W1, b1, W2, b2, W3, b3)

    in_maps = []
    for i in range(NCORES):
        xs = x[i * BSH : (i + 1) * BSH]
        m = dict(shared)
        for c in range(2):
            chain = xs[c * NB : (c + 1) * NB]  # [128, T, 51]
            arr = np.empty((KX, T, NB), bf)
            arr[:F] = chain.transpose(2, 1, 0).astype(bf)
            arr[F] = np.float32(1.0)
            m[f"x{c}"] = arr
        in_maps.append(m)

    trace = bool(os.environ.get("KLSTM_TRACE"))
    res = run_bass_kernel_spmd(nc, in_maps, list(range(NCORES)), trace=trace)
    LAST_RESULTS = res
    LAST_EXEC_NS = res.exec_time_ns
    out = np.concatenate([r["out"] for r in res.results], axis=0)
    return out.astype(np.float32)


# revision 16
# speedup vs baseline: 1.3826x; 1.2741x over previous
"""Trainium2 Bass kernel for the LSTM discriminator (fp8 DoubleRow version).

Model: LSTM(H=720) over x[B=2048, T=256, F=51], keep last hidden state,
then sigmoid -> Dense(1024) -> LeakyReLU(0.3) -> Dense(256) -> LeakyReLU(0.3)
-> Dense(1).

Strategy vs the bf16 baseline (which ran at the bf16 PE roofline, ~4.94ms):
  * Recurrence matmuls in fp8-e4m3 with MatmulPerfMode.DoubleRow: one
    instruction contracts TWO 128-row K-tiles (2x bf16 throughput on the PE).
    DoubleRow only pays off when the moving operand is wide, so z is computed
    *batch-major*: out[z^T] = [128 batch, 2880 gates], stationary = fp8
    h-state pairs [128, 2, 128], moving = fp8 Wh pairs [128, 2, <=512].
  * Quantization: Wh as e4m3(64*Wh), h as e4m3(16*h); the x projection stays
    bf16 (x-path quantization dominated the fp8 error budget) with weights
    pre-scaled by 1024 so both parts share one PSUM accumulation; the gate
    activation applies scale=1/1024. Measured end-to-end rel err ~3e-3.
  * x is pre-transposed on the host to [52, T, 128] bf16 per chain (51
    features + a ones-row that carries the LSTM bias), so the inner loop has
    no x transposes at all.
  * Two interleaved chains of 128 batch rows per core; z PSUM is 3 shared
    [128, 1024] tiles (6 banks) alternating between chains - ACT ops are
    tile-aligned so the write-after-read pacing is tile-granular.
  * Gate order in z columns is [i, f, o, g]: one sigmoid span (2160 cols) and
    one tanh span (720). Pointwise work is spread over ACT (gates + tanh(c)),
    GPSIMD (f*c, i*g) and DVE (c update, h=o*tanh(c), fp8 h-transpose copy).
  * h goes back to feature-major for the next step's stationary via 6 PE
    transposes (bf16, one PSUM bank), emitted mid-way through the *next*
    slot's matmul stream so the PE never waits on the pointwise chain.
  * fp32 head (sigmoid -> 3 dense layers), ~30us, after the loop.
"""

import os
import sys

import numpy as np

_TRN = "/opt/trn_rl_repo"
if _TRN not in sys.path:
    sys.path.insert(0, _TRN)

import ml_dtypes  # noqa: E402

import concourse.bacc as bacc  # noqa: E402
import concourse.tile as tile  # noqa: E402
from concourse import mybir  # noqa: E402
from concourse.bass_utils import run_bass_kernel_spmd  # noqa: E402

F32 = mybir.dt.float32
BF16 = mybir.dt.bfloat16
FP8 = mybir.dt.float8e4
AF = mybir.ActivationFunctionType
ALU = mybir.AluOpType
PM = mybir.MatmulPerfMode.DoubleRow

B, T_FULL, F, H = 2048, 256, 51, 720
D1, D2 = 1024, 256
NCORES = 8
BSH = B // NCORES   # 256 batch rows per core
NB = 128            # batch rows per chain (2 chains per core)
KX = F + 1          # x rows + ones row (bias)
G4 = 4 * H          # 2880
NP = 6              # h feature planes of 128 (720 -> 5x128 + 80, zero-padded)
TC = 32             # timesteps of x staged per DMA chunk
TPR = 4             # z-range index where the prev slot's h transposes go
S_W = 64.0          # Wh fp8 scale
S_H = 16.0          # h fp8 scale
S_Z = S_W * S_H     # total z scale (x-path weights pre-scaled by this)

# z column ranges (gate order g, i, f, o)
# tanh(g): cols [0, 720), sigmoid(i,f,o): cols [720, 2880)
# sg gate tile holds [i | f | o] at cols [0, 2160)
ZW = [512, 512, 512, 512, 512, 320]   # 6 ranges over 3 [128,1024] tiles

_NC_CACHE = {}
LAST_EXEC_NS = None
LAST_RESULTS = None


def _build(T):
    nc = bacc.Bacc(
        "TRN2", target_bir_lowering=False, debug=False, enable_asserts=False
    )

    xd = [
        nc.dram_tensor(f"x{c}", [KX, T, NB], BF16, kind="ExternalInput").ap()
        for c in range(2)
    ]
    whp_d = nc.dram_tensor("whp", [128, 3, 2, G4], FP8, kind="ExternalInput").ap()
    wxb_d = nc.dram_tensor("wxb", [KX, G4], BF16, kind="ExternalInput").ap()
    w1_d = nc.dram_tensor("w1", [128, NP, D1], F32, kind="ExternalInput").ap()
    w2_d = nc.dram_tensor("w2", [128, 8, D2], F32, kind="ExternalInput").ap()
    w3_d = nc.dram_tensor("w3", [128, 2, 1], F32, kind="ExternalInput").ap()
    b1_d = nc.dram_tensor("b1t", [128, 8], F32, kind="ExternalInput").ap()
    b2_d = nc.dram_tensor("b2t", [128, 2], F32, kind="ExternalInput").ap()
    b3_d = nc.dram_tensor("b3t", [1, 1], F32, kind="ExternalInput").ap()
    idb_d = nc.dram_tensor("identb", [128, 128], BF16, kind="ExternalInput").ap()
    idf_d = nc.dram_tensor("identf", [128, 128], F32, kind="ExternalInput").ap()
    out_d = nc.dram_tensor("out", [BSH, 1], F32, kind="ExternalOutput").ap()

    import contextlib

    with tile.TileContext(nc) as tc, contextlib.ExitStack() as ctx:
        if T > 16:
            tc.race_detector_enabled = False

        const = ctx.enter_context(tc.tile_pool(name="const", bufs=1))
        xpool = ctx.enter_context(tc.tile_pool(name="xp", bufs=2))
        gpool = ctx.enter_context(tc.tile_pool(name="gp", bufs=2))
        spool = ctx.enter_context(tc.tile_pool(name="st", bufs=1))
        zpool = ctx.enter_context(tc.tile_pool(name="zp", bufs=1, space="PSUM"))
        tpool = ctx.enter_context(tc.tile_pool(name="tp", bufs=1, space="PSUM"))

        # ---- constants / weights ----
        idb_t = const.tile([128, 128], BF16, tag="idb", name="idb")
        nc.sync.dma_start(idb_t[:], idb_d[:])
        idf_t = const.tile([128, 128], F32, tag="idf", name="idf")
        nc.sync.dma_start(idf_t[:], idf_d[:])
        whp_t = const.tile([128, 3, 2, G4], FP8, tag="whp", name="whp")
        wxb_t = const.tile([KX, G4], BF16, tag="wxb", name="wxb")
        w1_t = const.tile([128, NP, D1], F32, tag="w1", name="w1")
        w2_t = const.tile([128, 8, D2], F32, tag="w2", name="w2")
        w3_t = const.tile([128, 2, 1], F32, tag="w3", name="w3")
        b1_t = const.tile([128, 8], F32, tag="b1", name="b1")
        b2_t = const.tile([128, 2], F32, tag="b2", name="b2")
        b3_t = const.tile([1, 1], F32, tag="b3", name="b3")

        def load_weights():
            qs = [nc.gpsimd, nc.scalar, nc.sync]
            qi = [0]

            def dma(dst, src):
                qs[qi[0] % len(qs)].dma_start(dst, src)
                qi[0] += 1

            for k in range(NP):
                dma(w1_t[:, k, :], w1_d[:, k, :])
            for k in range(8):
                dma(w2_t[:, k, :], w2_d[:, k, :])
            dma(w3_t[:], w3_d[:])
            dma(b1_t[:], b1_d[:])
            dma(b2_t[:], b2_d[:])
            dma(b3_t[:], b3_d[:])
            dma(wxb_t[:], wxb_d[:])
            # whp last: the warmup matmuls read it, so they start right after
            # the weight DMAs and ramp the PE clock before the loop
            for p in range(3):
                dma(whp_t[:, p, :, :], whp_d[:, p, :, :])

        # ---- persistent state ----
        # hTq: fp8 16*h, feature-major planes [128 feat, plane, 128 batch]
        hTq = [spool.tile([128, NP, NB], FP8, tag=f"hTq{c}", name=f"hTq{c}") for c in range(2)]
        cS = [spool.tile([NB, H], F32, tag=f"c{c}", name=f"c{c}") for c in range(2)]
        # gates (f32, batch-major): sig = [i | f | o], tg = tanh(g)
        sig = [spool.tile([NB, 2160], F32, tag=f"sg{c}", name=f"sg{c}") for c in range(2)]
        tg = [spool.tile([NB, H], F32, tag=f"tg{c}", name=f"tg{c}") for c in range(2)]
        tcl = [spool.tile([NB, H], BF16, tag=f"tc{c}", name=f"tc{c}") for c in range(2)]
        t1g = [spool.tile([NB, H], F32, tag=f"t1{c}", name=f"t1{c}") for c in range(2)]
        t2g = [spool.tile([NB, H], F32, tag=f"t2{c}", name=f"t2{c}") for c in range(2)]
        hb = [spool.tile([NB, H], BF16, tag=f"hb{c}", name=f"hb{c}") for c in range(2)]
        for c in range(2):
            nc.vector.memset(hTq[c][:], 0.0)
            nc.vector.memset(cS[c][:], 0.0)

        # shared z PSUM: 3 tiles [128, 1024] = 6 banks, alternating chains
        zt = [zpool.tile([NB, 1024], F32, tag=f"z{i}", name=f"z{i}") for i in range(3)]
        # h-transpose landing bank (bf16, 6 planes); rows >= 80 of plane 5 are
        # never written by the transposes - zero once so the hTq copy reads 0s
        # (fp8 garbage there could be NaN patterns, and NaN*0 = NaN in PSUM)
        tpt = tpool.tile([128, NP, NB], BF16, tag="tpt", name="tpt")
        nc.vector.memset(tpt[:].bitcast(F32), 0.0)

        # ---- x chunks ----
        nchunks = (T + TC - 1) // TC
        xtiles = [[None] * nchunks for _ in range(2)]

        def ensure_chunk(c, ch):
            if ch >= nchunks or xtiles[c][ch] is not None:
                return
            sz = min(TC, T - ch * TC)
            t_ = xpool.tile([KX, TC, NB], BF16, tag=f"xc{c}", name=f"xc{c}")
            nc.sync.dma_start(t_[:, :sz, :], xd[c][:, ch * TC : ch * TC + sz, :])
            xtiles[c][ch] = t_

        ensure_chunk(0, 0)
        ensure_chunk(1, 0)
        load_weights()

        # ---- HAM warmup: PE busy ~5us right after the whp DMA lands ----
        for w_ in range(10):
            nc.tensor.matmul(
                zt[0][:, 0:512],
                hTq[0][:, 0:2, :],
                whp_t[:, 2, :, 2368:2880],
                start=True,
                stop=True,
                perf_mode=PM,
            )

        def emit_htranspose(s):
            # transpose h of slot s (bf16) into tpt, then fp8*16 into hTq
            c = s % 2
            for j in range(NP):
                w = min(128, H - 128 * j)
                nc.tensor.transpose(
                    tpt[0:w, j, :], hb[c][:, 128 * j : 128 * j + w], idb_t[:]
                )
            nc.vector.tensor_scalar_mul(hTq[c][:], tpt[:], S_H)

        def emit_z(s):
            c, t = s % 2, s // 2
            ch, toff = t // TC, t % TC
            if toff == 0:
                ensure_chunk(c, ch + 1)
            xsl = xtiles[c][ch][:, toff, :]

            def zrange(r):
                return zt[r // 2][:, (r % 2) * 512 : (r % 2) * 512 + ZW[r]]

            # all 6 x matmuls first (one stationary), then the DR chains
            col = 0
            for r in range(6):
                nc.tensor.matmul(
                    zrange(r), xsl, wxb_t[:, col : col + ZW[r]], start=True, stop=False
                )
                col += ZW[r]
            col = 0
            for r in range(6):
                out = zrange(r)
                for p in range(3):
                    nc.tensor.matmul(
                        out,
                        hTq[c][:, 2 * p : 2 * p + 2, :],
                        whp_t[:, p, :, col : col + ZW[r]],
                        start=False,
                        stop=(p == 2),
                        perf_mode=PM,
                    )
                col += ZW[r]
                if r == 3 and s >= 1 and (s - 1) // 2 < T - 1:
                    emit_htranspose(s - 1)

        def emit_pointwise(s):
            c, t = s % 2, s // 2
            # gates: z tiles -> SBUF, descaled by 1/S_Z
            # z cols [g | i | f | o]; sg = [i | f | o]
            nc.scalar.activation(tg[c][:], zt[0][:, 0:720], AF.Tanh, scale=1.0 / S_Z)
            nc.scalar.activation(
                sig[c][:, 0:304], zt[0][:, 720:1024], AF.Sigmoid, scale=1.0 / S_Z
            )
            nc.scalar.activation(
                sig[c][:, 304:1328], zt[1][:], AF.Sigmoid, scale=1.0 / S_Z
            )
            nc.scalar.activation(
                sig[c][:, 1328:2160], zt[2][:, 0:832], AF.Sigmoid, scale=1.0 / S_Z
            )
            # cell update: t2 = i*g (gpsimd, starts early), t1 = f*c (DVE)
            nc.gpsimd.tensor_mul(t2g[c][:], sig[c][:, 0:720], tg[c][:])
            nc.vector.tensor_mul(t1g[c][:], sig[c][:, 720:1440], cS[c][:])
            HH = H // 2
            nc.vector.tensor_add(cS[c][:, 0:HH], t1g[c][:, 0:HH], t2g[c][:, 0:HH])
            nc.vector.tensor_add(cS[c][:, HH:H], t1g[c][:, HH:H], t2g[c][:, HH:H])
            nc.scalar.activation(tcl[c][:, 0:HH], cS[c][:, 0:HH], AF.Tanh)
            nc.scalar.activation(tcl[c][:, HH:H], cS[c][:, HH:H], AF.Tanh)
            if t < T - 1:
                nc.vector.tensor_mul(hb[c][:], sig[c][:, 1440:2160], tcl[c][:])

        S = 2 * T
        for s in range(S):
            emit_z(s)
            emit_pointwise(s)

        # ---- head ----
        sgT = spool.tile([128, NP, BSH], F32, tag="sgT", name="sgT")
        hf = spool.tile([NB, H], F32, tag="hf", name="hf")
        for c in range(2):
            nc.vector.tensor_mul(hf[:], sig[c][:, 1440:2160], tcl[c][:])
            nc.scalar.activation(hf[:], hf[:], AF.Sigmoid)
            for j in range(NP):
                w = min(128, H - 128 * j)
                nc.tensor.transpose(
                    zt[0].bitcast(F32)[0:w, 128 * j : 128 * j + 128],
                    hf[:, 128 * j : 128 * j + w],
                    idf_t[:],
                )
            nc.vector.tensor_copy(
                sgT[:, :, c * NB : c * NB + NB],
                zt[0][:, 0:768],
            )

        o1 = spool.tile([128, 8, BSH], F32, tag="o1", name="o1")
        for m in range(8):
            ps = zt[1][:, 0:BSH]
            for j in range(NP):
                nc.tensor.matmul(
                    ps,
                    w1_t[:, j, m * 128 : (m + 1) * 128],
                    sgT[:, j, :],
                    start=(j == 0),
                    stop=(j == NP - 1),
                )
            tb = gpool.tile([128, BSH], F32, tag="hb2", name="hb2")
            nc.vector.tensor_scalar_add(tb[:], ps, b1_t[:, m : m + 1])
            nc.vector.scalar_tensor_tensor(
                o1[:, m, :], tb[:], 0.3, tb[:], ALU.mult, ALU.max
            )
        o2 = spool.tile([128, 2, BSH], F32, tag="o2", name="o2")
        for m in range(2):
            ps = zt[2][:, 0:BSH]
            for k in range(8):
                nc.tensor.matmul(
                    ps,
                    w2_t[:, k, m * 128 : (m + 1) * 128],
                    o1[:, k, :],
                    start=(k == 0),
                    stop=(k == 7),
                )
            tb = gpool.tile([128, BSH], F32, tag="hb2", name="hb2")
            nc.vector.tensor_scalar_add(tb[:], ps, b2_t[:, m : m + 1])
            nc.vector.scalar_tensor_tensor(
                o2[:, m, :], tb[:], 0.3, tb[:], ALU.mult, ALU.max
            )
        ps = zt[1][0:1, 512 : 512 + BSH]
        for k in range(2):
            nc.tensor.matmul(
                ps, w3_t[:, k, :], o2[:, k, :], start=(k == 0), stop=(k == 1)
            )
        ob = spool.tile([1, BSH], F32, tag="ob", name="ob")
        nc.vector.tensor_scalar_add(ob[:], ps, b3_t[:])
        nc.sync.dma_start(out_d[:], ob[:])

    nc.compile()
    return nc


def _get_nc(T):
    if T not in _NC_CACHE:
        _NC_CACHE[T] = _build(T)
    return _NC_CACHE[T]


def _prep_weights(Wx, Wh, b, W1, b1, W2, b2, W3, b3):
    f32 = np.float32
    bf = ml_dtypes.bfloat16
    e4 = ml_dtypes.float8_e4m3fn

    # gate reorder: reference z = [i | f | g | o] -> ours [g | i | f | o]
    def reord(w):
        return np.concatenate(
            [w[..., 1440:2160], w[..., 0:1440], w[..., 2160:2880]], axis=-1
        )

    Whr = reord(np.asarray(Wh, f32))
    Wxr = reord(np.asarray(Wx, f32))
    br = reord(np.asarray(b, f32).reshape(1, G4))[0]

    whp = np.zeros((128, 3, 2, G4), f32)
    for p in range(3):
        for q in range(2):
            r0 = 128 * (2 * p + q)
            r1 = min(r0 + 128, H)
            if r0 < H:
                whp[0 : r1 - r0, p, q, :] = Whr[r0:r1]
    whp = np.clip(whp * S_W, -240, 240).astype(e4)

    wxb = np.zeros((KX, G4), f32)
    wxb[:F] = Wxr
    wxb[F] = br
    wxb = (wxb * S_Z).astype(bf)

    w1 = np.zeros((128, NP, D1), f32)
    W1a = np.asarray(W1, f32)
    for j in range(NP):
        r0 = 128 * j
        r1 = min(r0 + 128, H)
        w1[0 : r1 - r0, j, :] = W1a[r0:r1]
    w2 = np.ascontiguousarray(np.asarray(W2, f32).reshape(8, 128, D2).transpose(1, 0, 2))
    w3 = np.ascontiguousarray(np.asarray(W3, f32).reshape(2, 128, 1).transpose(1, 0, 2))
    b1t = np.ascontiguousarray(np.asarray(b1, f32).reshape(8, 128).T)
    b2t = np.ascontiguousarray(np.asarray(b2, f32).reshape(2, 128).T)
    b3t = np.asarray(b3, f32).reshape(1, 1)
    return {
        "whp": whp,
        "wxb": wxb,
        "w1": w1,
        "w2": w2,
        "w3": w3,
        "b1t": b1t,
        "b2t": b2t,
        "b3t": b3t,
        "identb": np.eye(128, dtype=bf),
        "identf": np.eye(128, dtype=f32),
    }


def kernel(x, Wx, Wh, b, W1, b1, W2, b2, W3, b3):
    global LAST_EXEC_NS, LAST_RESULTS
    x = np.asarray(x, dtype=np.float32)
    T = x.shape[1]
    nc = _get_nc(T)
    bf = ml_dtypes.bfloat16

    shared = _prep_weights(Wx, Wh, b, W1, b1, W2, b2, W3, b3)

    in_maps = []
    for i in range(NCORES):
        xs = x[i * BSH : (i + 1) * BSH]
        m = dict(shared)
        for c in range(2):
            chain = xs[c * NB : (c + 1) * NB]  # [128, T, 51]
            arr = np.empty((KX, T, NB), bf)
            arr[:F] = chain.transpose(2, 1, 0).astype(bf)
            arr[F] = np.float32(1.0)
            m[f"x{c}"] = arr
        in_maps.append(m)

    trace = bool(os.environ.get("KLSTM_TRACE"))
    res = run_bass_kernel_spmd(nc, in_maps, list(range(NCORES)), trace=trace)
    LAST_RESULTS = res
    LAST_EXEC_NS = res.exec_time_ns
    out = np.concatenate([r["out"] for r in res.results], axis=0)
    return out.astype(np.float32)
